# revision 21
# baseline (speedup 1.0000x reference)
"""Trainium2 Bass kernel for AttnDecoderRNN single step (batch=1).

8-way tensor parallel, ONE on-device collective (AllGather, 512B/rank fp16):
  - attention fully replicated per core: 3 output strips at PSUM partitions
    {0,32,64} via PE column tiling; softmax runs multi-lane on the strips
  - context computed in full on every core (encoder_outputs replicated),
    weights-stationary so it lands directly in column layout
  - combine (2048x4096) ROW-sharded: each core computes its exact 256-slot
    slice of g = relu(comb_W @ [x;ctx] + b) locally -> AllGather of g (fp16)
  - GRU row-sharded over output slots; w_hh@h + all biases accumulate into
    PSUM during the collective-wait window (keeps the PE warm), w_ih@g joins
    the same accumulation group afterwards, gates run on 4-lane [4,64] views
  - final gather of h_new slices done on host; attn weights from core 0

All matmul operands are fp16 (single-pass PE, half HBM traffic); PSUM
accumulation is fp32.  Weights stream as the MOVING operand with the input
vector chunk stationary ([128,1] lhsT).  M=1 GEMVs are spread across PE
column strips (tile_position=(0,32j)) so strips run concurrently and every
per-strip result lands at its own PSUM partition - no single-lane row math.
Biases enter via K=1 matmuls (lhsT = [1,1] one) instead of vector adds.
The g vector uses an interleaved chunk layout (chunk k = indices {16p+k});
GRU weight rows/cols are permuted on the host to match.
"""

import numpy as np

import concourse.bacc as bacc
import concourse.bass as bass
import concourse.mybir as mybir
import concourse.tile as tile
from concourse import bass_utils

F32 = mybir.dt.float32
F16 = mybir.dt.float16
NP16 = np.float16
NCORES = 8
H = 2048          # hidden size
L = 350           # max_length
LP = 384          # padded max_length (3 x 128)
HC = H // 128     # 16 column chunks of h / g
ZC = 2 * HC       # 32 column chunks of [x; h]
GS = 3 * (H // NCORES)   # 768 GRU rows per core (r,z,n x 256)
CS = H // NCORES  # 256: per-core slice of x / ctx / h_new

NEG16 = -60000.0  # fp16-safe softmax pad (exp -> 0)

_CACHE = {}


def _build():
    nc = bacc.Bacc(
        "TRN2",
        target_bir_lowering=False,
        debug=False,
        enable_asserts=True,
        num_devices=NCORES,
    )
    rg = [list(range(NCORES))]

    # ---- external inputs (per-core data prepared on host) ----
    d_attn_wt = nc.dram_tensor("attn_wt", [128, ZC * LP], F16, kind="ExternalInput")
    d_attn_b = nc.dram_tensor("attn_b", [1, LP], F16, kind="ExternalInput")
    d_z_cols = nc.dram_tensor("z_cols", [128, ZC], F16, kind="ExternalInput")
    d_enc = nc.dram_tensor("enc_full", [LP, H], F16, kind="ExternalInput")
    d_comb_wt = nc.dram_tensor("comb_wt", [128, ZC * CS], F16, kind="ExternalInput")
    d_comb_b = nc.dram_tensor("comb_b", [1, CS], F16, kind="ExternalInput")
    d_wih = nc.dram_tensor("wih_t", [H, GS], F16, kind="ExternalInput")
    d_whh = nc.dram_tensor("whh_t", [H, GS], F16, kind="ExternalInput")
    d_h_cols = nc.dram_tensor("h_cols", [128, HC], F16, kind="ExternalInput")
    d_hsl = nc.dram_tensor("hsl", [4, 64], F32, kind="ExternalInput")
    d_bias_main = nc.dram_tensor("bias_main", [1, GS], F16, kind="ExternalInput")
    d_bias_hn = nc.dram_tensor("bias_hn", [1, CS], F16, kind="ExternalInput")

    # ---- external outputs ----
    d_h_part = nc.dram_tensor("h_part", [4, 64], F32, kind="ExternalOutput")
    d_aw = nc.dram_tensor("aw_full", [3, 128], F32, kind="ExternalOutput")

    ACT = mybir.ActivationFunctionType
    AX = mybir.AxisListType

    with tile.TileContext(nc) as tc:
        with (
            tc.tile_pool(name="wts", bufs=1) as wp,
            tc.tile_pool(name="work", bufs=1) as wk,
            tc.tile_pool(name="psum", bufs=1, space="PSUM") as ps,
            tc.tile_pool(name="dram", bufs=1, space="DRAM") as dram,
        ):
            # ---------- weight / input DMAs (issue order sets priority) ----------
            z_cols = wp.tile([128, ZC], F16)
            nc.sync.dma_start(z_cols[:], d_z_cols[:])
            attn_b = wp.tile([1, LP], F16)
            nc.sync.dma_start(attn_b[:], d_attn_b[:])
            one1 = wp.tile([1, 1], F16)
            nc.vector.memset(one1[:], 1.0)
            # attention weights: one tile per k-chunk
            attn_w = []
            for k in range(ZC):
                t = wp.tile([128, LP], F16, name=f"attn_w{k}")
                nc.sync.dma_start(t[:], d_attn_wt[:, k * LP:(k + 1) * LP])
                attn_w.append(t)
            enc_sb = []
            for k in range(3):
                t = wp.tile([128, H], F16, name=f"enc_{k}")
                nc.sync.dma_start(t[:, :H // 2], d_enc[k * 128:(k + 1) * 128, :H // 2])
                nc.sync.dma_start(t[:, H // 2:], d_enc[k * 128:(k + 1) * 128, H // 2:])
                enc_sb.append(t)
            comb_sb = []
            for k in range(ZC):
                t = wp.tile([128, CS], F16, name=f"comb_{k}")
                nc.sync.dma_start(t[:], d_comb_wt[:, k * CS:(k + 1) * CS])
                comb_sb.append(t)
            comb_b = wp.tile([1, CS], F16)
            nc.sync.dma_start(comb_b[:], d_comb_b[:])
            h_cols = wp.tile([128, HC], F16)
            nc.sync.dma_start(h_cols[:], d_h_cols[:])
            hsl = wp.tile([4, 64], F32)
            nc.sync.dma_start(hsl[:], d_hsl[:])
            whh_sb = []
            for k in range(HC):
                t = wp.tile([128, GS], F16, name=f"whh_{k}")
                nc.sync.dma_start(t[:], d_whh[k * 128:(k + 1) * 128, :])
                whh_sb.append(t)
            wih_sb = []
            for k in range(HC):
                t = wp.tile([128, GS], F16, name=f"wih_{k}")
                nc.sync.dma_start(t[:], d_wih[k * 128:(k + 1) * 128, :])
                wih_sb.append(t)
            bias_main = wp.tile([1, GS], F16)
            nc.sync.dma_start(bias_main[:], d_bias_main[:])
            bias_hn = wp.tile([1, CS], F16)
            nc.sync.dma_start(bias_hn[:], d_bias_hn[:])

            # ---------- attention logits (replicated): 3 slot strips ------------
            # strip s at PSUM partition 32s holds slots [128s, 128s+128)
            at_ps = ps.tile([128, 128], F32, tag="sp", bufs=1)
            for s in range(3):
                nc.tensor.matmul(
                    at_ps[32 * s:32 * s + 1, :], one1[:],
                    attn_b[0:1, s * 128:(s + 1) * 128],
                    start=True, stop=False, tile_position=(0, 32 * s),
                )
            for k in range(ZC):
                for s in range(3):
                    nc.tensor.matmul(
                        at_ps[32 * s:32 * s + 1, :], z_cols[:, k:k + 1],
                        attn_w[k][:, s * 128:(s + 1) * 128],
                        start=False, stop=(k == ZC - 1), tile_position=(0, 32 * s),
                    )
            # per-strip exp + row sums (single-partition ops at bases 0/32/64)
            expv = wk.tile([65, 128], F32)
            psum3 = wk.tile([65, 1], F32)
            for s in range(3):
                nc.scalar.activation(expv[32 * s:32 * s + 1, :],
                                     at_ps[32 * s:32 * s + 1, :], ACT.Exp)
                nc.vector.reduce_sum(psum3[32 * s:32 * s + 1, :],
                                     expv[32 * s:32 * s + 1, :], axis=AX.X)
            # DMA-pack the 3 sums, then a K=3 ones-matmul broadcasts the total
            sums3 = wk.tile([3, 1], F32)
            nc.sync.dma_start(sums3[:], psum3[0:65:32, :])
            sums16 = wk.tile([3, 1], F16)
            nc.vector.tensor_copy(sums16[:], sums3[:])
            ones3 = wp.tile([3, 128], F16)
            nc.vector.memset(ones3[:], 1.0)
            tb_ps = ps.tile([128, 1], F32, tag="sp2", bufs=1)
            nc.tensor.matmul(tb_ps[:], ones3[:], sums16[:], start=True, stop=True)
            rcpb = wk.tile([128, 1], F32)
            nc.vector.reciprocal(rcpb[:], tb_ps[:])
            # normalized attention weights: fp16 strips for compute + fp32 out
            aw16 = wk.tile([65, 128], F16)
            awf = wk.tile([65, 128], F32)
            for s in range(3):
                r = rcpb[32 * s:32 * s + 1, :]
                nc.vector.tensor_scalar_mul(aw16[32 * s:32 * s + 1, :],
                                            expv[32 * s:32 * s + 1, :], r)
                nc.vector.tensor_scalar_mul(awf[32 * s:32 * s + 1, :],
                                            expv[32 * s:32 * s + 1, :], r)
            nc.sync.dma_start(d_aw[:], awf[0:65:32, :])

            # transpose aw strips -> 3 column chunks via a small DRAM bounce
            aw_dram = dram.tile([3, 128], F16)
            nc.sync.dma_start(aw_dram[:], aw16[0:65:32, :])
            aw_cols = wk.tile([128, 3], F16)
            nc.sync.dma_start(aw_cols[:], aw_dram[:].rearrange("s p -> p s"))

            # ---------- full context as columns [128,16] (weights stationary) ----
            ctx_ps = ps.tile([128, HC], F32, tag="sp2", bufs=1)
            for m in range(HC):
                for k in range(3):
                    nc.tensor.matmul(
                        ctx_ps[:, m:m + 1], enc_sb[k][:, m * 128:(m + 1) * 128],
                        aw_cols[:, k:k + 1], start=(k == 0), stop=(k == 2),
                    )
            ctx = wk.tile([128, HC], F16)
            nc.vector.tensor_copy(ctx[:], ctx_ps[:])

            # ---------- combine row-shard: exact g slice, 4 output strips -------
            # strip j at partition 32j holds local g slots [64j, 64j+64)
            cb_ps = ps.tile([128, 64], F32, tag="cp", bufs=1)
            for j in range(4):
                nc.tensor.matmul(
                    cb_ps[32 * j:32 * j + 1, :], one1[:],
                    comb_b[0:1, j * 64:(j + 1) * 64],
                    start=True, stop=False, tile_position=(0, 32 * j),
                )
            for k in range(ZC):
                lhs = z_cols[:, k:k + 1] if k < HC else ctx[:, k - HC:k - HC + 1]
                for j in range(4):
                    nc.tensor.matmul(
                        cb_ps[32 * j:32 * j + 1, :], lhs,
                        comb_sb[k][:, j * 64:(j + 1) * 64],
                        start=False, stop=(k == ZC - 1), tile_position=(0, 32 * j),
                    )
            g_slice = wk.tile([97, 64], F16)
            for j in range(4):
                nc.scalar.activation(g_slice[32 * j:32 * j + 1, :],
                                     cb_ps[32 * j:32 * j + 1, :], ACT.Relu)

            # ---------- AllGather the finished g slices (fp16, 512B/rank) --------
            cc_in = dram.tile([4, 64], F16)
            cc_out = dram.tile([1, H], F16, addr_space="Shared")
            nc.sync.dma_start(cc_in[:], g_slice[0:97:32, :])
            nc.gpsimd.collective_compute(
                "AllGather", mybir.AluOpType.bypass, replica_groups=rg,
                ins=[cc_in[:]], outs=[cc_out[:]],
            )

            # ---------- GRU accumulation: w_hh@h + biases during the wait -------
            # main strip j at partition 32j: cols [r(64) | z(64) | n(64)] for
            # output slots [64j, 64j+64); hn strip j: h_n (w_hh n-part + b_hh_n)
            main_ps = ps.tile([128, 192], F32, tag="mp", bufs=1)
            hn_ps = ps.tile([128, 64], F32, tag="hp", bufs=1)
            for j in range(4):
                nc.tensor.matmul(
                    main_ps[32 * j:32 * j + 1, :], one1[:],
                    bias_main[0:1, j * 192:(j + 1) * 192],
                    start=True, stop=False, tile_position=(0, 32 * j),
                )
                nc.tensor.matmul(
                    hn_ps[32 * j:32 * j + 1, :], one1[:],
                    bias_hn[0:1, j * 64:(j + 1) * 64],
                    start=True, stop=False, tile_position=(0, 32 * j),
                )
            for k in range(HC):
                for j in range(4):
                    nc.tensor.matmul(
                        main_ps[32 * j:32 * j + 1, 0:128], h_cols[:, k:k + 1],
                        whh_sb[k][:, j * 192:j * 192 + 128],
                        start=False, stop=False, tile_position=(0, 32 * j),
                    )
                    nc.tensor.matmul(
                        hn_ps[32 * j:32 * j + 1, :], h_cols[:, k:k + 1],
                        whh_sb[k][:, j * 192 + 128:j * 192 + 192],
                        start=False, stop=(k == HC - 1), tile_position=(0, 32 * j),
                    )

            # load gathered g as [128,16] fp16 (chunk k = indices {16p+k})
            g16 = wk.tile([128, HC], F16)
            nc.sync.dma_start(g16[:], cc_out[0, :].rearrange("(p k) -> p k", k=HC))

            # ---------- w_ih@g joins the same accumulation group ----------------
            for k in range(HC):
                for j in range(4):
                    nc.tensor.matmul(
                        main_ps[32 * j:32 * j + 1, :], g16[:, k:k + 1],
                        wih_sb[k][:, j * 192:(j + 1) * 192],
                        start=False, stop=(k == HC - 1), tile_position=(0, 32 * j),
                    )

            # ---------- compact the strips to contiguous partitions --------------
            stage_m = wk.tile([97, 192], F32)
            stage_h = wk.tile([97, 64], F32)
            for j in range(4):
                sl = slice(32 * j, 32 * j + 1)
                if j % 2 == 0:
                    nc.vector.tensor_copy(stage_m[sl, :], main_ps[sl, :])
                    nc.scalar.copy(stage_h[sl, :], hn_ps[sl, :])
                else:
                    nc.scalar.copy(stage_m[sl, :], main_ps[sl, :])
                    nc.vector.tensor_copy(stage_h[sl, :], hn_ps[sl, :])
            cm = wk.tile([4, 192], F32)
            nc.sync.dma_start(cm[:], stage_m[0:97:32, :])
            ch = wk.tile([4, 64], F32)
            nc.sync.dma_start(ch[:], stage_h[0:97:32, :])

            # ---------- GRU gates on 4-lane [4,64] contiguous tiles --------------
            rzs = wk.tile([4, 128], F32)
            nc.scalar.activation(rzs[:], cm[:, 0:128], ACT.Sigmoid)
            t1 = wk.tile([4, 64], F32)
            nc.vector.tensor_mul(t1[:], rzs[:, 0:64], ch[:])
            t2 = wk.tile([4, 64], F32)
            nc.vector.tensor_add(t2[:], t1[:], cm[:, 128:192])
            nt = wk.tile([4, 64], F32)
            nc.scalar.activation(nt[:], t2[:], ACT.Tanh)
            hmn = wk.tile([4, 64], F32)
            nc.vector.tensor_sub(hmn[:], hsl[:], nt[:])
            zt = wk.tile([4, 64], F32)
            nc.vector.tensor_mul(zt[:], rzs[:, 64:128], hmn[:])
            hnew = wk.tile([4, 64], F32)
            nc.vector.tensor_add(hnew[:], nt[:], zt[:])
            nc.sync.dma_start(d_h_part[:], hnew[:])

    nc.compile()
    return nc


def _prep(inputs):
    """Build per-core input maps from the full problem inputs."""
    f = lambda a: np.ascontiguousarray(np.asarray(a, dtype=np.float32))
    x = f(inputs["input"]).reshape(H)
    h = f(inputs["hidden"]).reshape(H)
    enc = f(inputs["encoder_outputs"])
    attn_W = f(inputs["attn_W"])
    attn_b = f(inputs["attn_b"])
    comb_W = f(inputs["comb_W"])
    comb_b = f(inputs["comb_b"])
    w_ih = f(inputs["w_ih"])
    w_hh = f(inputs["w_hh"])
    b_ih = f(inputs["b_ih"])
    b_hh = f(inputs["b_hh"])

    z = np.concatenate([x, h])
    z_cols = np.ascontiguousarray(z.reshape(ZC, 128).T.astype(NP16))
    h_cols = np.ascontiguousarray(h.astype(NP16).reshape(128, HC))   # interleaved

    Wp = np.zeros((LP, 2 * H), np.float32)
    Wp[:L] = attn_W
    bp = np.full((1, LP), NEG16, np.float32)
    bp[0, :L] = attn_b
    encp16 = np.zeros((LP, H), NP16)
    encp16[:L] = enc.astype(NP16)
    # replicated attention weights, packed for k-chunked rhs access
    attn_wt = np.ascontiguousarray(
        Wp.T.reshape(ZC, 128, LP).transpose(1, 0, 2).reshape(128, ZC * LP).astype(NP16))
    # row permutation so k-chunk k of the GRU contraction = g indices {16p+k}
    perm = np.add.outer(np.arange(HC), HC * np.arange(128)).reshape(-1)

    in_maps = []
    for c in range(NCORES):
        # GRU columns per strip j: [r(64) | z(64) | n(64)] for slots 64j..64j+64
        sel = np.concatenate([
            np.arange(c * CS + 64 * j, c * CS + 64 * (j + 1)) + gate * H
            for j in range(4) for gate in range(3)])
        bsum = b_ih[sel] + b_hh[sel]
        bias_main = bsum.copy()
        # n-part of main carries only b_ih
        for j in range(4):
            bias_main[192 * j + 128:192 * j + 192] = b_ih[sel[192 * j + 128:192 * j + 192]]
        bias_hn = np.concatenate([
            b_hh[sel[192 * j + 128:192 * j + 192]] for j in range(4)])
        cwt = comb_W[c * CS:(c + 1) * CS].T.astype(NP16)  # (4096, 256)
        comb_wt = np.ascontiguousarray(
            cwt.reshape(ZC, 128, CS).transpose(1, 0, 2).reshape(128, ZC * CS))
        in_maps.append({
            "attn_wt": attn_wt,
            "attn_b": bp.astype(NP16),
            "z_cols": z_cols,
            "enc_full": encp16,
            "comb_wt": comb_wt,
            "comb_b": np.ascontiguousarray(comb_b[c * CS:(c + 1) * CS].reshape(1, CS).astype(NP16)),
            "wih_t": np.ascontiguousarray(w_ih[sel].T[perm].astype(NP16)),
            "whh_t": np.ascontiguousarray(w_hh[sel].T[perm].astype(NP16)),
            "h_cols": h_cols,
            "hsl": np.ascontiguousarray(h[c * CS:(c + 1) * CS].reshape(4, 64)),
            "bias_main": np.ascontiguousarray(bias_main.reshape(1, GS).astype(NP16)),
            "bias_hn": np.ascontiguousarray(bias_hn.reshape(1, CS).astype(NP16)),
        })
    return in_maps


def kernel(**inputs):
    if "nc" not in _CACHE:
        _CACHE["nc"] = _build()
    nc = _CACHE["nc"]
    in_maps = _prep(inputs)
    res = bass_utils.run_bass_kernel_spmd(
        nc, in_maps, core_ids=list(range(NCORES)), **_CACHE.get("run_kwargs", {}))
    _CACHE["last_result"] = res

    h_full = np.concatenate(
        [np.asarray(res.results[c]["h_part"]).reshape(CS) for c in range(NCORES)])
    aw_full = np.asarray(res.results[0]["aw_full"]).reshape(LP)[:L]
    out = h_full.reshape(1, 1, H).astype(np.float32)
    return (out, out.copy(), aw_full.reshape(1, L).astype(np.float32))


# revision 22
# speedup vs baseline: 1.3255x; 1.3255x over previous
"""Trainium2 Bass kernel for AttnDecoderRNN single step (batch=1).

8-way tensor parallel, ONE on-device collective (AllGather, 512B/rank fp16):
  - attention fully replicated per core: 3 output strips at PSUM partitions
    {0,32,64} via PE column tiling; softmax runs multi-lane on the strips
  - context computed in full on every core (encoder_outputs replicated),
    weights-stationary so it lands directly in column layout
  - combine (2048x4096) ROW-sharded: each core computes its exact 256-slot
    slice of g = relu(comb_W @ [x;ctx] + b) locally -> AllGather of g (fp16)
  - GRU row-sharded over output slots; w_hh@h + all biases accumulate into
    PSUM during the collective-wait window (keeps the PE warm), w_ih@g joins
    the same accumulation group afterwards, gates run on 4-lane [4,64] views
  - final gather of h_new slices done on host; attn weights from core 0

All matmul operands are fp16 (single-pass PE, half HBM traffic); PSUM
accumulation is fp32.  Weights stream as the MOVING operand with the input
vector chunk stationary ([128,1] lhsT).  M=1 GEMVs are spread across PE
column strips (tile_position=(0,32j)) so strips run concurrently and every
per-strip result lands at its own PSUM partition - no single-lane row math.
Biases enter via K=1 matmuls (lhsT = [1,1] one) instead of vector adds.
The g vector uses an interleaved chunk layout (chunk k = indices {16p+k});
GRU weight rows/cols are permuted on the host to match.
"""

import numpy as np

import concourse.bacc as bacc
import concourse.bass as bass
import concourse.mybir as mybir
import concourse.tile as tile
from concourse import bass_utils

F32 = mybir.dt.float32
F16 = mybir.dt.float16
NP16 = np.float16
NCORES = 8
H = 2048          # hidden size
L = 350           # max_length
LP = 384          # padded max_length (3 x 128)
HC = H // 128     # 16 column chunks of h / g
ZC = 2 * HC       # 32 column chunks of [x; h]
GS = 3 * (H // NCORES)   # 768 GRU rows per core (r,z,n x 256)
CS = H // NCORES  # 256: per-core slice of x / ctx / h_new

NEG16 = -60000.0  # fp16-safe softmax pad (exp -> 0)

_CACHE = {}


def _build():
    nc = bacc.Bacc(
        "TRN2",
        target_bir_lowering=False,
        debug=False,
        enable_asserts=True,
        num_devices=NCORES,
    )
    rg = [list(range(NCORES))]

    # ---- external inputs (per-core data prepared on host) ----
    d_attn_wt = nc.dram_tensor("attn_wt", [128, ZC * LP], F16, kind="ExternalInput")
    d_attn_b = nc.dram_tensor("attn_b", [1, LP], F16, kind="ExternalInput")
    d_z_cols = nc.dram_tensor("z_cols", [128, ZC], F16, kind="ExternalInput")
    d_enc = nc.dram_tensor("enc_full", [LP, H], F16, kind="ExternalInput")
    d_comb_wt = nc.dram_tensor("comb_wt", [128, ZC * CS], F16, kind="ExternalInput")
    d_comb_b = nc.dram_tensor("comb_b", [1, CS], F16, kind="ExternalInput")
    d_wih = nc.dram_tensor("wih_t", [H, GS], F16, kind="ExternalInput")
    d_whh = nc.dram_tensor("whh_t", [H, GS], F16, kind="ExternalInput")
    d_h_cols = nc.dram_tensor("h_cols", [128, HC], F16, kind="ExternalInput")
    d_hsl = nc.dram_tensor("hsl", [4, 64], F32, kind="ExternalInput")
    d_bias_main = nc.dram_tensor("bias_main", [1, GS], F16, kind="ExternalInput")
    d_bias_hn = nc.dram_tensor("bias_hn", [1, CS], F16, kind="ExternalInput")

    # ---- external outputs ----
    d_h_part = nc.dram_tensor("h_part", [4, 64], F32, kind="ExternalOutput")
    d_aw = nc.dram_tensor("aw_full", [3, 128], F32, kind="ExternalOutput")

    ACT = mybir.ActivationFunctionType
    AX = mybir.AxisListType

    with tile.TileContext(nc) as tc:
        with (
            tc.tile_pool(name="wts", bufs=1) as wp,
            tc.tile_pool(name="work", bufs=1) as wk,
            tc.tile_pool(name="psum", bufs=1, space="PSUM") as ps,
            tc.tile_pool(name="dram", bufs=1, space="DRAM") as dram,
        ):
            # ---------- weight / input DMAs (issue order sets priority) ----------
            z_cols = wp.tile([128, ZC], F16)
            nc.sync.dma_start(z_cols[:], d_z_cols[:])
            attn_b = wp.tile([1, LP], F16)
            nc.sync.dma_start(attn_b[:], d_attn_b[:])
            one1 = wp.tile([1, 1], F16)
            nc.vector.memset(one1[:], 1.0)
            # attention weights: one tile per k-chunk
            attn_w = []
            for k in range(ZC):
                t = wp.tile([128, LP], F16, name=f"attn_w{k}")
                nc.sync.dma_start(t[:], d_attn_wt[:, k * LP:(k + 1) * LP])
                attn_w.append(t)
            enc_sb = []
            for k in range(3):
                t = wp.tile([128, H], F16, name=f"enc_{k}")
                nc.sync.dma_start(t[:, :H // 2], d_enc[k * 128:(k + 1) * 128, :H // 2])
                nc.sync.dma_start(t[:, H // 2:], d_enc[k * 128:(k + 1) * 128, H // 2:])
                enc_sb.append(t)
            comb_sb = []
            for k in range(ZC):
                t = wp.tile([128, CS], F16, name=f"comb_{k}")
                nc.sync.dma_start(t[:], d_comb_wt[:, k * CS:(k + 1) * CS])
                comb_sb.append(t)
            comb_b = wp.tile([1, CS], F16)
            nc.sync.dma_start(comb_b[:], d_comb_b[:])
            h_cols = wp.tile([128, HC], F16)
            nc.sync.dma_start(h_cols[:], d_h_cols[:])
            hsl = wp.tile([4, 64], F32)
            nc.sync.dma_start(hsl[:], d_hsl[:])
            whh_sb = []
            for k in range(HC):
                t = wp.tile([128, GS], F16, name=f"whh_{k}")
                nc.sync.dma_start(t[:], d_whh[k * 128:(k + 1) * 128, :])
                whh_sb.append(t)
            wih_sb = []
            for k in range(HC):
                t = wp.tile([128, GS], F16, name=f"wih_{k}")
                nc.sync.dma_start(t[:], d_wih[k * 128:(k + 1) * 128, :])
                wih_sb.append(t)
            bias_main = wp.tile([1, GS], F16)
            nc.sync.dma_start(bias_main[:], d_bias_main[:])
            bias_hn = wp.tile([1, CS], F16)
            nc.sync.dma_start(bias_hn[:], d_bias_hn[:])

            # ---------- attention logits (replicated): 3 slot strips ------------
            # strip s at PSUM partition 32s holds slots [128s, 128s+128)
            at_ps = ps.tile([128, 128], F32, tag="sp", bufs=1)
            for s in range(3):
                nc.tensor.matmul(
                    at_ps[32 * s:32 * s + 1, :], one1[:],
                    attn_b[0:1, s * 128:(s + 1) * 128],
                    start=True, stop=False, tile_position=(0, 32 * s),
                )
            for k in range(ZC):
                for s in range(3):
                    nc.tensor.matmul(
                        at_ps[32 * s:32 * s + 1, :], z_cols[:, k:k + 1],
                        attn_w[k][:, s * 128:(s + 1) * 128],
                        start=False, stop=(k == ZC - 1), tile_position=(0, 32 * s),
                    )
            # per-strip exp + row sums (single-partition ops at bases 0/32/64)
            expv = wk.tile([65, 128], F32)
            psum3 = wk.tile([65, 1], F32)
            for s in range(3):
                nc.scalar.activation(expv[32 * s:32 * s + 1, :],
                                     at_ps[32 * s:32 * s + 1, :], ACT.Exp)
                nc.vector.reduce_sum(psum3[32 * s:32 * s + 1, :],
                                     expv[32 * s:32 * s + 1, :], axis=AX.X)
            # DMA-pack the 3 sums, then a K=3 ones-matmul broadcasts the total
            sums3 = wk.tile([3, 1], F32)
            nc.gpsimd.dma_start(sums3[:], psum3[0:65:32, :])
            sums16 = wk.tile([3, 1], F16)
            nc.vector.tensor_copy(sums16[:], sums3[:])
            ones3 = wp.tile([3, 128], F16)
            nc.vector.memset(ones3[:], 1.0)
            tb_ps = ps.tile([128, 1], F32, tag="sp2", bufs=1)
            nc.tensor.matmul(tb_ps[:], ones3[:], sums16[:], start=True, stop=True)
            rcpb = wk.tile([128, 1], F32)
            nc.vector.reciprocal(rcpb[:], tb_ps[:])
            # normalized attention weights: fp16 strips for compute + fp32 out
            aw16 = wk.tile([65, 128], F16)
            awf = wk.tile([65, 128], F32)
            for s in range(3):
                r = rcpb[32 * s:32 * s + 1, :]
                nc.vector.tensor_scalar_mul(aw16[32 * s:32 * s + 1, :],
                                            expv[32 * s:32 * s + 1, :], r)
                nc.vector.tensor_scalar_mul(awf[32 * s:32 * s + 1, :],
                                            expv[32 * s:32 * s + 1, :], r)
            nc.gpsimd.dma_start(d_aw[:], awf[0:65:32, :])

            # transpose aw strips -> 3 column chunks via a small DRAM bounce
            aw_dram = dram.tile([3, 128], F16)
            nc.gpsimd.dma_start(aw_dram[:], aw16[0:65:32, :])
            aw_cols = wk.tile([128, 3], F16)
            nc.gpsimd.dma_start(aw_cols[:], aw_dram[:].rearrange("s p -> p s"))

            # ---------- full context as columns [128,16] (weights stationary) ----
            ctx_ps = ps.tile([128, HC], F32, tag="sp2", bufs=1)
            for m in range(HC):
                for k in range(3):
                    nc.tensor.matmul(
                        ctx_ps[:, m:m + 1], enc_sb[k][:, m * 128:(m + 1) * 128],
                        aw_cols[:, k:k + 1], start=(k == 0), stop=(k == 2),
                    )
            ctx = wk.tile([128, HC], F16)
            nc.vector.tensor_copy(ctx[:], ctx_ps[:])

            # ---------- combine row-shard: exact g slice, 4 output strips -------
            # strip j at partition 32j holds local g slots [64j, 64j+64)
            cb_ps = ps.tile([128, 64], F32, tag="cp", bufs=1)
            for j in range(4):
                nc.tensor.matmul(
                    cb_ps[32 * j:32 * j + 1, :], one1[:],
                    comb_b[0:1, j * 64:(j + 1) * 64],
                    start=True, stop=False, tile_position=(0, 32 * j),
                )
            for k in range(ZC):
                lhs = z_cols[:, k:k + 1] if k < HC else ctx[:, k - HC:k - HC + 1]
                for j in range(4):
                    nc.tensor.matmul(
                        cb_ps[32 * j:32 * j + 1, :], lhs,
                        comb_sb[k][:, j * 64:(j + 1) * 64],
                        start=False, stop=(k == ZC - 1), tile_position=(0, 32 * j),
                    )
            g_slice = wk.tile([97, 64], F16)
            for j in range(4):
                nc.scalar.activation(g_slice[32 * j:32 * j + 1, :],
                                     cb_ps[32 * j:32 * j + 1, :], ACT.Relu)

            # ---------- AllGather the finished g slices (fp16, 512B/rank) --------
            cc_in = dram.tile([4, 64], F16)
            cc_out = dram.tile([1, H], F16, addr_space="Shared")
            nc.gpsimd.dma_start(cc_in[:], g_slice[0:97:32, :])
            nc.gpsimd.collective_compute(
                "AllGather", mybir.AluOpType.bypass, replica_groups=rg,
                ins=[cc_in[:]], outs=[cc_out[:]],
            )

            # ---------- GRU accumulation: w_hh@h + biases during the wait -------
            # main strip j at partition 32j: cols [r(64) | z(64) | n(64)] for
            # output slots [64j, 64j+64); hn strip j: h_n (w_hh n-part + b_hh_n)
            main_ps = ps.tile([128, 192], F32, tag="mp", bufs=1)
            hn_ps = ps.tile([128, 64], F32, tag="hp", bufs=1)
            for j in range(4):
                nc.tensor.matmul(
                    main_ps[32 * j:32 * j + 1, :], one1[:],
                    bias_main[0:1, j * 192:(j + 1) * 192],
                    start=True, stop=False, tile_position=(0, 32 * j),
                )
                nc.tensor.matmul(
                    hn_ps[32 * j:32 * j + 1, :], one1[:],
                    bias_hn[0:1, j * 64:(j + 1) * 64],
                    start=True, stop=False, tile_position=(0, 32 * j),
                )
            for k in range(HC):
                for j in range(4):
                    nc.tensor.matmul(
                        main_ps[32 * j:32 * j + 1, 0:128], h_cols[:, k:k + 1],
                        whh_sb[k][:, j * 192:j * 192 + 128],
                        start=False, stop=False, tile_position=(0, 32 * j),
                    )
                    nc.tensor.matmul(
                        hn_ps[32 * j:32 * j + 1, :], h_cols[:, k:k + 1],
                        whh_sb[k][:, j * 192 + 128:j * 192 + 192],
                        start=False, stop=(k == HC - 1), tile_position=(0, 32 * j),
                    )

            # load gathered g as [128,16] fp16 (chunk k = indices {16p+k})
            g16 = wk.tile([128, HC], F16)
            nc.gpsimd.dma_start(g16[:], cc_out[0, :].rearrange("(p k) -> p k", k=HC))

            # ---------- w_ih@g joins the same accumulation group ----------------
            for k in range(HC):
                for j in range(4):
                    nc.tensor.matmul(
                        main_ps[32 * j:32 * j + 1, :], g16[:, k:k + 1],
                        wih_sb[k][:, j * 192:(j + 1) * 192],
                        start=False, stop=(k == HC - 1), tile_position=(0, 32 * j),
                    )

            # ---------- compact the strips to contiguous partitions --------------
            stage_m = wk.tile([97, 192], F32)
            stage_h = wk.tile([97, 64], F32)
            for j in range(4):
                sl = slice(32 * j, 32 * j + 1)
                if j % 2 == 0:
                    nc.vector.tensor_copy(stage_m[sl, :], main_ps[sl, :])
                    nc.scalar.copy(stage_h[sl, :], hn_ps[sl, :])
                else:
                    nc.scalar.copy(stage_m[sl, :], main_ps[sl, :])
                    nc.vector.tensor_copy(stage_h[sl, :], hn_ps[sl, :])
            cm = wk.tile([4, 192], F32)
            nc.gpsimd.dma_start(cm[:], stage_m[0:97:32, :])
            ch = wk.tile([4, 64], F32)
            nc.gpsimd.dma_start(ch[:], stage_h[0:97:32, :])

            # ---------- GRU gates on 4-lane [4,64] contiguous tiles --------------
            rzs = wk.tile([4, 128], F32)
            nc.scalar.activation(rzs[:], cm[:, 0:128], ACT.Sigmoid)
            t1 = wk.tile([4, 64], F32)
            nc.vector.tensor_mul(t1[:], rzs[:, 0:64], ch[:])
            t2 = wk.tile([4, 64], F32)
            nc.vector.tensor_add(t2[:], t1[:], cm[:, 128:192])
            nt = wk.tile([4, 64], F32)
            nc.scalar.activation(nt[:], t2[:], ACT.Tanh)
            hmn = wk.tile([4, 64], F32)
            nc.vector.tensor_sub(hmn[:], hsl[:], nt[:])
            zt = wk.tile([4, 64], F32)
            nc.vector.tensor_mul(zt[:], rzs[:, 64:128], hmn[:])
            hnew = wk.tile([4, 64], F32)
            nc.vector.tensor_add(hnew[:], nt[:], zt[:])
            nc.gpsimd.dma_start(d_h_part[:], hnew[:])

    nc.compile()
    return nc


def _prep(inputs):
    """Build per-core input maps from the full problem inputs."""
    f = lambda a: np.ascontiguousarray(np.asarray(a, dtype=np.float32))
    x = f(inputs["input"]).reshape(H)
    h = f(inputs["hidden"]).reshape(H)
    enc = f(inputs["encoder_outputs"])
    attn_W = f(inputs["attn_W"])
    attn_b = f(inputs["attn_b"])
    comb_W = f(inputs["comb_W"])
    comb_b = f(inputs["comb_b"])
    w_ih = f(inputs["w_ih"])
    w_hh = f(inputs["w_hh"])
    b_ih = f(inputs["b_ih"])
    b_hh = f(inputs["b_hh"])

    z = np.concatenate([x, h])
    z_cols = np.ascontiguousarray(z.reshape(ZC, 128).T.astype(NP16))
    h_cols = np.ascontiguousarray(h.astype(NP16).reshape(128, HC))   # interleaved

    Wp = np.zeros((LP, 2 * H), np.float32)
    Wp[:L] = attn_W
    bp = np.full((1, LP), NEG16, np.float32)
    bp[0, :L] = attn_b
    encp16 = np.zeros((LP, H), NP16)
    encp16[:L] = enc.astype(NP16)
    # replicated attention weights, packed for k-chunked rhs access
    attn_wt = np.ascontiguousarray(
        Wp.T.reshape(ZC, 128, LP).transpose(1, 0, 2).reshape(128, ZC * LP).astype(NP16))
    # row permutation so k-chunk k of the GRU contraction = g indices {16p+k}
    perm = np.add.outer(np.arange(HC), HC * np.arange(128)).reshape(-1)

    in_maps = []
    for c in range(NCORES):
        # GRU columns per strip j: [r(64) | z(64) | n(64)] for slots 64j..64j+64
        sel = np.concatenate([
            np.arange(c * CS + 64 * j, c * CS + 64 * (j + 1)) + gate * H
            for j in range(4) for gate in range(3)])
        bsum = b_ih[sel] + b_hh[sel]
        bias_main = bsum.copy()
        # n-part of main carries only b_ih
        for j in range(4):
            bias_main[192 * j + 128:192 * j + 192] = b_ih[sel[192 * j + 128:192 * j + 192]]
        bias_hn = np.concatenate([
            b_hh[sel[192 * j + 128:192 * j + 192]] for j in range(4)])
        cwt = comb_W[c * CS:(c + 1) * CS].T.astype(NP16)  # (4096, 256)
        comb_wt = np.ascontiguousarray(
            cwt.reshape(ZC, 128, CS).transpose(1, 0, 2).reshape(128, ZC * CS))
        in_maps.append({
            "attn_wt": attn_wt,
            "attn_b": bp.astype(NP16),
            "z_cols": z_cols,
            "enc_full": encp16,
            "comb_wt": comb_wt,
            "comb_b": np.ascontiguousarray(comb_b[c * CS:(c + 1) * CS].reshape(1, CS).astype(NP16)),
            "wih_t": np.ascontiguousarray(w_ih[sel].T[perm].astype(NP16)),
            "whh_t": np.ascontiguousarray(w_hh[sel].T[perm].astype(NP16)),
            "h_cols": h_cols,
            "hsl": np.ascontiguousarray(h[c * CS:(c + 1) * CS].reshape(4, 64)),
            "bias_main": np.ascontiguousarray(bias_main.reshape(1, GS).astype(NP16)),
            "bias_hn": np.ascontiguousarray(bias_hn.reshape(1, CS).astype(NP16)),
        })
    return in_maps


def kernel(**inputs):
    if "nc" not in _CACHE:
        _CACHE["nc"] = _build()
    nc = _CACHE["nc"]
    in_maps = _prep(inputs)
    res = bass_utils.run_bass_kernel_spmd(
        nc, in_maps, core_ids=list(range(NCORES)), **_CACHE.get("run_kwargs", {}))
    _CACHE["last_result"] = res

    h_full = np.concatenate(
        [np.asarray(res.results[c]["h_part"]).reshape(CS) for c in range(NCORES)])
    aw_full = np.asarray(res.results[0]["aw_full"]).reshape(LP)[:L]
    out = h_full.reshape(1, 1, H).astype(np.float32)
    return (out, out.copy(), aw_full.reshape(1, L).astype(np.float32))
